# revision 30
# baseline (speedup 1.0000x reference)
"""MLA attention (DeepSeek-style) distributed over 8 TRN2 NeuronCores.

Sharding: core c -> batch b=c//4, head-group/seq-shard g=c%4.
Phase A: down-proj of own 512-pos shard -> bf16 bounce -> group-of-4
AllGathers (kv first, then q split in two, so comms overlap projection
compute). kv latent is pre-scaled by inv-rms before the bounce.
Phase B: up-proj (Qt/Kt d-major, V row-major), flash-style causal attention
with St[kv,q] layout (softmax denominators via ones-matmul, no transposes).
Attention outputs exchanged per head via an 8-rank AllToAll (each rank's
shard j carries the o block destined for rank j); the row-parallel wo
matmul on the own s-shard is interleaved between attention heads and
accumulated in SBUF.
"""

import numpy as np
import ml_dtypes

import concourse.bass as bass
import concourse.bacc as bacc
import concourse.tile as tile
import concourse.mybir as mybir
from concourse.bass_utils import run_bass_kernel_spmd

BF16 = ml_dtypes.bfloat16

# problem constants (hardcoded per harness rules)
DIM = 2048
N_HEADS = 16
Q_LORA = 1536
KV_LORA = 512
NOPE = 128
ROPE = 64
V_DIM = 128
QK_HD = NOPE + ROPE  # 192
EPS = 1e-6
B, S = 2, 2048
SCALE = QK_HD ** -0.5

NCORES = 8
GROUP = 4               # cores per batch
SSH = S // GROUP        # 512, seq shard
HPC = N_HEADS // GROUP  # 4 heads per core
P = 128
NKT = DIM // P          # 16
NQM = Q_LORA // P       # 12
NKVM = KV_LORA // P     # 4
NCH = S // 512          # 4
KVROWS = KV_LORA + ROPE       # 576: pre-scaled kv latent + rope'd k_pe
                              # (k_pe is ready early, and the last q AG gates
                              # the critical path, so it rides with kv)
QROWS = NQM * P + 1           # 1537: q latent + a_q row. One AG: the chain
                              # is serial with ~15us per-op start gaps and
                              # data is ready before the chain reaches it,
                              # so one op beats two equal pieces

_cache = {}


def _build():
    nc = bacc.Bacc("TRN2", target_bir_lowering=False, debug=False,
                   num_devices=NCORES)
    f32 = mybir.dt.float32
    bf = mybir.dt.bfloat16
    i32 = mybir.dt.int32

    # ---- dram parameters ----
    xT = nc.dram_tensor("xT", [DIM, SSH], bf, kind="ExternalInput")
    wqaT = nc.dram_tensor("wqaT", [DIM, Q_LORA], bf, kind="ExternalInput")
    wkvaT = nc.dram_tensor("wkvaT", [DIM, KV_LORA + ROPE], bf,
                           kind="ExternalInput")
    wqbT = nc.dram_tensor("wqbT", [Q_LORA, HPC * QK_HD], bf,
                          kind="ExternalInput")
    wkvbT = nc.dram_tensor("wkvbT", [KV_LORA, HPC * (NOPE + V_DIM)], bf,
                           kind="ExternalInput")
    # wo.T rows regrouped head-major: rows [h*512 + g'*128 ...] = head g'*4+h
    woTr = nc.dram_tensor("woTr", [N_HEADS * V_DIM, DIM], bf,
                          kind="ExternalInput")
    cos_sh = nc.dram_tensor("cos_sh", [P, SSH], bf, kind="ExternalInput")
    sin_sh = nc.dram_tensor("sin_sh", [P, SSH], bf, kind="ExternalInput")
    cos_full = nc.dram_tensor("cos_full", [P, S], bf, kind="ExternalInput")
    sin_full = nc.dram_tensor("sin_full", [P, S], bf, kind="ExternalInput")
    perm64 = nc.dram_tensor("perm64", [P, P], bf, kind="ExternalInput")
    trimask = nc.dram_tensor("trimask", [P, P], f32, kind="ExternalInput")
    cfg = nc.dram_tensor("cfg", [1, 1], i32, kind="ExternalInput")
    outT = nc.dram_tensor("out", [DIM, SSH], f32, kind="ExternalOutput")

    # ---- internal dram ----
    b_kv = nc.dram_tensor("b_kv", [KVROWS, SSH], bf)
    g_kv = nc.dram_tensor("g_kv", [GROUP * KVROWS, SSH], bf)
    b_q = nc.dram_tensor("b_q", [QROWS, SSH], bf)
    g_q = nc.dram_tensor("g_q", [GROUP * QROWS, SSH], bf)
    # AllToAll o exchange: shard j (rows j*128..) goes to rank j. Only the
    # own-batch shards carry data; other-batch shards are never read.
    o_in = [nc.dram_tensor(f"o_in{h}", [NCORES * V_DIM, SSH], bf)
            for h in range(HPC)]
    o_out = [nc.dram_tensor(f"o_out{h}", [NCORES * V_DIM, SSH], bf)
             for h in range(HPC)]
    rg4 = [[0, 1, 2, 3], [4, 5, 6, 7]]
    rg8 = [list(range(NCORES))]

    with tile.TileContext(nc) as tc:
        with (
            tc.tile_pool(name="persist", bufs=1) as persist,
            tc.tile_pool(name="attn", bufs=1) as attn_pool,
            tc.tile_pool(name="wts", bufs=1) as wts,
        ):
            # constants (cheap, engine-local)
            ones_f = persist.tile([P, 1], f32)
            nc.vector.memset(ones_f, 1.0)
            ones_b = persist.tile([P, 1], bf)
            nc.vector.memset(ones_b, 1.0)
            ones_row = persist.tile([1, P], bf)
            nc.vector.memset(ones_row, 1.0)
            eps_sb = persist.tile([1, 1], f32)
            nc.vector.memset(eps_sb, EPS)
            cfg_sb = persist.tile([1, 1], i32)
            nc.sync.dma_start(out=cfg_sb, in_=cfg[:])

            # per-core dynamic offset: ob = b*512 (A2A shard base)
            r0 = nc.alloc_registers()
            nc.regs_load(r0, cfg_sb[0:1, 0:1])
            ob = nc.snap(r0, donate=True, min_val=0, max_val=GROUP * P)

            # attention-phase persistent tiles (filled by up-proj)
            qt_nope = [attn_pool.tile([P, S], bf, tag=f"qtn{h}",
                                      name=f"qt_nope{h}") for h in range(HPC)]
            qt_pe = [attn_pool.tile([P, S], bf, tag=f"qtp{h}",
                                    name=f"qt_pe{h}")
                     for h in range(HPC // 2)]
            kt_nope = [attn_pool.tile([P, S], bf, tag=f"ktn{h}",
                                      name=f"kt_nope{h}") for h in range(HPC)]
            v_all = attn_pool.tile([P, S // P, HPC * V_DIM], bf)
            kpe_dup = attn_pool.tile([P, NCH, 512], bf)

            # weights/tables prefetched on the gpsimd SWDGE queue (idle
            # early) so they don't delay phase-A HWDGE traffic
            cos_sh_sb = persist.tile([P, SSH], bf)
            nc.gpsimd.dma_start(out=cos_sh_sb, in_=cos_sh[:])
            sin_sh_sb = persist.tile([P, SSH], bf)
            nc.gpsimd.dma_start(out=sin_sh_sb, in_=sin_sh[:])
            perm_sb = persist.tile([P, P], bf)
            nc.gpsimd.dma_start(out=perm_sb, in_=perm64[:])
            wkvb = wts.tile([P, NKVM, HPC * (NOPE + V_DIM)], bf)
            nc.gpsimd.dma_start(
                out=wkvb, in_=wkvbT[:].rearrange("(kt p) m -> p kt m", p=P))
            wqb = wts.tile([P, NQM, HPC * QK_HD], bf)
            nc.gpsimd.dma_start(
                out=wqb, in_=wqbT[:].rearrange("(kt p) m -> p kt m", p=P))
            mask_sb = persist.tile([P, P], f32)
            nc.gpsimd.dma_start(out=mask_sb, in_=trimask[:])
            cos_f_sb = persist.tile([P, S], bf)
            nc.gpsimd.dma_start(out=cos_f_sb, in_=cos_full[:])
            sin_f_sb = persist.tile([P, S], bf)
            nc.gpsimd.dma_start(out=sin_f_sb, in_=sin_full[:])

            # ======== Phase A + up-proj (shared latent pool) ========
            up_lat_cm = tc.tile_pool(name="up_lat", bufs=1)
            up_lat = up_lat_cm.__enter__()
            with (
                tc.tile_pool(name="pa", bufs=2) as pa,
                tc.tile_pool(name="pa_x", bufs=1) as pa_x,
                tc.tile_pool(name="pa_out", bufs=3) as pa_out,
                tc.tile_pool(name="pa_ps", bufs=2, space="PSUM") as pa_ps,
                tc.tile_pool(name="pa_st", bufs=1, space="PSUM") as pa_st,
            ):
                # 8-way split so the first slab's k-loop chases x arrival
                x_all = pa_x.tile([P, NKT, SSH], bf)
                for xq in range(8):
                    nc.sync.dma_start(
                        out=x_all[:, xq * 2:(xq + 1) * 2, :],
                        in_=xT[xq * 2 * P:(xq + 1) * 2 * P, :].rearrange(
                            "(kt p) s -> p kt s", p=P))

                q_stat = pa_st.tile([1, SSH], f32)
                kv_stat = pa_st.tile([1, SSH], f32)

                def down_slab(wT, m0, mrows, ev, stat_ps, stat_first,
                              stat_last):
                    slab = pa.tile([P, NKT, mrows], bf, tag="slab")
                    nc.scalar.dma_start(
                        out=slab,
                        in_=wT[:, m0:m0 + mrows].rearrange(
                            "(kt p) m -> p kt m", p=P))
                    ps = pa_ps.tile([P, SSH], f32, tag="dps")
                    for k in range(NKT):
                        nc.tensor.matmul(ps[:mrows, :], slab[:, k, :],
                                         x_all[:, k, :], start=(k == 0),
                                         stop=(k == NKT - 1))
                    nc.vector.tensor_copy(ev[:mrows, :], ps[:mrows, :])
                    if stat_ps is not None:
                        # square on the vector engine (from the SBUF copy:
                        # DVE reads at most one PSUM operand): the scalar
                        # queue must stay pure-DMA-dispatch or slab loads
                        # serialize behind psum-dependent compute
                        sq = pa.tile([P, SSH], f32, tag="sq")
                        nc.vector.tensor_mul(sq[:mrows, :], ev[:mrows, :],
                                             ev[:mrows, :])
                        nc.tensor.matmul(stat_ps, ones_f[:mrows, :],
                                         sq[:mrows, :], start=stat_first,
                                         stop=stat_last)

                def inv_rms(stat, n):
                    tmp = pa.tile([1, SSH], f32, tag="srt")
                    nc.scalar.activation(tmp, stat,
                                         mybir.ActivationFunctionType.Sqrt,
                                         bias=eps_sb[0:1, 0:1], scale=1.0 / n)
                    rcp = pa.tile([1, SSH], f32, tag="rcp")
                    nc.vector.reciprocal(rcp, tmp)
                    rb = pa.tile([1, SSH], bf, tag="rb")
                    nc.vector.tensor_copy(rb, rcp)
                    return rb

                # ---- kv first (so AG_kv overlaps q down-proj) ----
                kv_ev = pa_x.tile([P, NKVM, SSH], bf)
                for m in range(NKVM):
                    down_slab(wkvaT, m * P, P, kv_ev[:, m, :], kv_stat,
                              m == 0, m == NKVM - 1)
                # rope'd k_pe rides in the q AllGather (not needed until
                # attention), keeping AG_kv minimal on the critical path
                kpe_ev = pa_out.tile([P, SSH], bf, tag="kpe_ev")
                down_slab(wkvaT, KV_LORA, ROPE, kpe_ev, None, False, False)
                xs_ps = pa_ps.tile([ROPE, SSH], f32, tag="xs")
                nc.tensor.matmul(xs_ps, perm_sb[:ROPE, :ROPE], kpe_ev[:ROPE, :])
                y0 = pa.tile([ROPE, SSH], bf, tag="ry0")
                nc.vector.tensor_mul(y0, kpe_ev[:ROPE, :], cos_sh_sb[:ROPE, :])
                y1 = pa.tile([ROPE, SSH], bf, tag="ry1")
                nc.vector.tensor_mul(y1, xs_ps, sin_sh_sb[:ROPE, :])
                yr = pa.tile([ROPE, SSH], bf, tag="ryr")
                nc.vector.tensor_add(yr, y0, y1)
                nc.sync.dma_start(out=b_kv[KV_LORA:KVROWS, :], in_=yr)
                # first q slabs keep the tensor stream dense while the kv
                # stat -> scale -> bounce chain resolves
                for m in range(2):
                    ev = pa_out.tile([P, SSH], bf, tag="qev")
                    down_slab(wqaT, m * P, P, ev, q_stat, m == 0, False)
                    nc.sync.dma_start(out=b_q[m * P:(m + 1) * P, :], in_=ev)
                # pre-scale kv latent by inv-rms, then bounce. The
                # partition-broadcast of the inv-rms row is a K=1 matmul
                # (ones column outer product) -- the gpsimd broadcast costs
                # a lib load plus ~15us of false semaphore waits here.
                rkv = inv_rms(kv_stat, KV_LORA)
                rkv_ps = pa_ps.tile([P, SSH], f32, tag="rkb")
                nc.tensor.matmul(rkv_ps, ones_row, rkv)
                for m in range(NKVM):
                    nc.vector.tensor_mul(kv_ev[:, m, :], kv_ev[:, m, :],
                                         rkv_ps)
                    nc.sync.dma_start(out=b_kv[m * P:(m + 1) * P, :],
                                      in_=kv_ev[:, m, :])

                nc.gpsimd.collective_compute(
                    "AllGather", mybir.AluOpType.bypass, replica_groups=rg4,
                    ins=[b_kv[:]], outs=[g_kv[:]])

                # kv gather loads on the gpsimd queue right behind the AG_kv
                # trigger: gpsimd SWDGE uses its own semaphore lanes, so
                # these AG-gated loads cannot falsely delay the HWDGE
                # bounce-write lanes that gate the later AG triggers
                kv_lat = up_lat.tile([P, NKVM, NCH, 512], bf)
                for r in range(NCH):
                    nc.gpsimd.dma_start(
                        out=kv_lat[:, :, r, :],
                        in_=g_kv[r * KVROWS:r * KVROWS + KV_LORA, :]
                        .rearrange("(kt p) s -> p kt s", p=P))

                # ---- remaining q down-proj (overlaps AG_kv) ----
                for m in range(2, NQM):
                    ev = pa_out.tile([P, SSH], bf, tag="qev")
                    down_slab(wqaT, m * P, P, ev, q_stat, False,
                              m == NQM - 1)
                    nc.sync.dma_start(out=b_q[m * P:(m + 1) * P, :], in_=ev)
                rq = inv_rms(q_stat, Q_LORA)
                nc.sync.dma_start(out=b_q[NQM * P:QROWS, :], in_=rq)
                nc.gpsimd.collective_compute(
                    "AllGather", mybir.AluOpType.bypass, replica_groups=rg4,
                    ins=[b_q[:]], outs=[g_q[:]])

                # kpe / a_q gather loads, also on the gpsimd lanes
                aq_row = up_lat.tile([1, NCH, 512], bf)
                for r in range(NCH):
                    nc.gpsimd.dma_start(
                        out=kpe_dup[:ROPE, r, :],
                        in_=g_kv[r * KVROWS + KV_LORA:(r + 1) * KVROWS, :])
                    nc.gpsimd.dma_start(
                        out=kpe_dup[ROPE:, r, :],
                        in_=g_kv[r * KVROWS + KV_LORA:(r + 1) * KVROWS, :])
                    nc.gpsimd.dma_start(
                        out=aq_row[0:1, r, :],
                        in_=g_q[r * QROWS + NQM * P:(r + 1) * QROWS, :])
                a_q_bc = up_lat.tile([P, NCH, 512], bf)
                for r in range(NCH):
                    nc.gpsimd.partition_broadcast(a_q_bc[:, r, :],
                                                  aq_row[0:1, r, :])

            # ================= Phase B: up projections =================
            with (
                tc.tile_pool(name="up", bufs=3) as up,
                tc.tile_pool(name="qlat", bufs=2) as qlat_pool,
                tc.tile_pool(name="up_ps", bufs=3, space="PSUM") as up_ps,
                tc.tile_pool(name="pe_ps", bufs=2, space="PSUM") as pe_ps,
            ):
                # k_nope (d-major) + v (row-major); kv_lat is pre-scaled
                for c in range(NCH):
                    for h in range(HPC):
                        ps = up_ps.tile([P, 512], f32, tag="up")
                        for k in range(NKVM):
                            nc.tensor.matmul(
                                ps, wkvb[:, k, h * NOPE:(h + 1) * NOPE],
                                kv_lat[:, k, c, :], start=(k == 0),
                                stop=(k == NKVM - 1))
                        nc.vector.tensor_copy(
                            kt_nope[h][:, c * 512:(c + 1) * 512], ps)
                for sb in range(S // P):
                    c, part = sb // 4, sb % 4
                    ps = up_ps.tile([P, HPC * V_DIM], f32, tag="up")
                    for k in range(NKVM):
                        nc.tensor.matmul(
                            ps, kv_lat[:, k, c, part * P:(part + 1) * P],
                            wkvb[:, k, HPC * NOPE:], start=(k == 0),
                            stop=(k == NKVM - 1))
                    nc.vector.tensor_copy(v_all[:, sb, :], ps)

                # ---- q up-proj (waits on AG_q1/2; q_lat streamed) ----
                for c in range(NCH):
                    ql = qlat_pool.tile([P, NQM, 512], bf, tag="ql",
                                        name="ql")
                    nc.sync.dma_start(
                        out=ql,
                        in_=g_q[c * QROWS:c * QROWS + NQM * P, :]
                        .rearrange("(kt p) s -> p kt s", p=P))
                    for h in range(HPC):
                        ps = up_ps.tile([P, 512], f32, tag="up")
                        for k in range(NQM):
                            nc.tensor.matmul(
                                ps, wqb[:, k, h * P:(h + 1) * P],
                                ql[:, k, :], start=(k == 0),
                                stop=(k == NQM - 1))
                        nc.vector.tensor_mul(
                            qt_nope[h][:, c * 512:(c + 1) * 512], ps,
                            a_q_bc[:, c, :])
                    for hp in range(HPC // 2):
                        pcol0 = HPC * NOPE + 2 * hp * ROPE
                        ps = pe_ps.tile([P, 512], f32, tag="qp")
                        for k in range(NQM):
                            nc.tensor.matmul(
                                ps, wqb[:, k, pcol0:pcol0 + 2 * ROPE],
                                ql[:, k, :], start=(k == 0),
                                stop=(k == NQM - 1))
                        pe_s = up.tile([P, 512], bf, tag="pes")
                        nc.vector.tensor_mul(pe_s, ps, a_q_bc[:, c, :])
                        xs = pe_ps.tile([P, 512], f32, tag="qpx")
                        nc.tensor.matmul(xs, perm_sb, pe_s)
                        dst = qt_pe[hp][:, c * 512:(c + 1) * 512]
                        nc.vector.tensor_mul(
                            dst, pe_s, cos_f_sb[:, c * 512:(c + 1) * 512])
                        t1 = up.tile([P, 512], bf, tag="pet")
                        nc.vector.tensor_mul(
                            t1, xs, sin_f_sb[:, c * 512:(c + 1) * 512])
                        nc.vector.tensor_add(dst, dst, t1)

            up_lat_cm.__exit__(None, None, None)

            # ========== attention + per-head A2As + interleaved wo ==========
            with (
                tc.tile_pool(name="at", bufs=3) as at,
                tc.tile_pool(name="at_rl", bufs=2) as at_rl,
                tc.tile_pool(name="wo_rhs", bufs=2) as wo_rhs,
                tc.tile_pool(name="wo_acc", bufs=1) as wo_acc,
                tc.tile_pool(name="wo_w", bufs=2) as wo_w,
                tc.tile_pool(name="wo_ev", bufs=3) as wo_ev,
                tc.tile_pool(name="st_ps", bufs=2, space="PSUM") as st_ps,
                tc.tile_pool(name="ot_ps", bufs=2, space="PSUM") as ot_ps,
                tc.tile_pool(name="l_ps", bufs=2, space="PSUM") as l_ps,
                tc.tile_pool(name="wo_ps", bufs=2, space="PSUM") as wo_ps,
            ):
                acc = wo_acc.tile([P, NKT, 512], f32)

                def attention_head(h):
                    pending = None  # (pj, off, j, ot, lt, first, last)

                    def flush():
                        nonlocal pending
                        if pending is None:
                            return
                        pj, off, j, ot, lt, first, last = pending
                        nc.tensor.matmul(lt[:, off:], ones_b, pj[:, off:],
                                         start=first, stop=last)
                        nc.tensor.matmul(
                            ot[:, off:],
                            v_all[:, j, h * V_DIM:(h + 1) * V_DIM],
                            pj[:, off:], start=first, stop=last)
                        pending = None

                    def evict(ot, lt, qc):
                        rl = at_rl.tile([1, 512], f32, tag="rl", name="rl")
                        nc.vector.reciprocal(rl, lt)
                        rlb = at_rl.tile([P, 512], f32, tag="rlb",
                                         name="rlb")
                        nc.gpsimd.partition_broadcast(rlb, rl)
                        ev = at.tile([P, 512], bf, tag="oev", name="oev")
                        nc.vector.tensor_mul(ev, ot, rlb)
                        nc.sync.dma_start(
                            out=o_in[h][bass.ds(ob + qc * P, P), :],
                            in_=ev)

                    # prefetch this head's wo weights (no deps)
                    wslab = wo_w.tile([P, GROUP, DIM], bf, tag="woslab",
                                      name="wslab")
                    nc.scalar.dma_start(
                        out=wslab,
                        in_=woTr[h * 512:(h + 1) * 512, :].rearrange(
                            "(kt p) m -> p kt m", p=P))

                    evs = []
                    for qc in range(NCH):
                        nj = qc * 4 + 4
                        ot = ot_ps.tile([P, 512], f32, tag="ot", name="ot")
                        lt = l_ps.tile([1, 512], f32, tag="l", name="lt")
                        for j in range(nj):
                            d = j - qc * 4
                            off = max(0, d) * P
                            st = st_ps.tile([P, 512], f32, tag="st",
                                            name="st")
                            nc.tensor.matmul(
                                st[:, off:],
                                kt_nope[h][:, j * P:(j + 1) * P],
                                qt_nope[h][:, qc * 512 + off:(qc + 1) * 512],
                                start=True, stop=False)
                            lo = (h % 2) * ROPE
                            nc.tensor.matmul(
                                st[:, off:],
                                kpe_dup[lo:lo + ROPE, j // 4,
                                        (j % 4) * P:(j % 4 + 1) * P],
                                qt_pe[h // 2][lo:lo + ROPE,
                                              qc * 512 + off:(qc + 1) * 512],
                                start=False, stop=True)
                            flush()
                            if j == 0 and evs:
                                # previous chunk's accumulation closed with
                                # the flush above: evict it now so the o
                                # exchange isn't serialized at head end
                                evict(*evs.pop())
                            if d >= 0:
                                nc.vector.tensor_add(st[:, off:off + P],
                                                     st[:, off:off + P],
                                                     mask_sb)
                            pj = at.tile([P, 512], bf, tag="p", name="pj")
                            nc.scalar.activation(
                                pj[:, off:], st[:, off:],
                                mybir.ActivationFunctionType.Exp)
                            pending = (pj, off, j, ot, lt, j == 0,
                                       j == nj - 1)
                        evs.append((ot, lt, qc))
                    flush()
                    evict(*evs.pop())
                    nc.gpsimd.collective_compute(
                        "AllToAll", mybir.AluOpType.bypass,
                        replica_groups=rg8, ins=[o_in[h][:]],
                        outs=[o_out[h][:]])
                    # rhs loads for the wo pass of this head (gpsimd queue,
                    # blocked only by this A2A)
                    rhs = wo_rhs.tile([P, GROUP, 512], bf, tag="rhs",
                                      name="rhs")
                    for k in range(GROUP):
                        nc.gpsimd.dma_start(
                            out=rhs[:, k, :],
                            in_=o_out[h][bass.ds(ob + k * P, P), :])
                    return rhs, wslab

                def wo_pass(h, rhs, wslab):
                    for m in range(NKT):
                        ps = wo_ps.tile([P, 512], f32, tag="wops",
                                        name="wops")
                        for k in range(GROUP):
                            nc.tensor.matmul(
                                ps, wslab[:, k, m * P:(m + 1) * P],
                                rhs[:, k, :], start=(k == 0),
                                stop=(k == GROUP - 1))
                        if h == 0:
                            nc.vector.tensor_copy(acc[:, m, :], ps)
                        elif h < HPC - 1:
                            nc.vector.tensor_add(acc[:, m, :], ps,
                                                 acc[:, m, :])
                        else:
                            ev = wo_ev.tile([P, 512], f32, tag="woev",
                                            name="woev")
                            nc.vector.tensor_add(ev, ps, acc[:, m, :])
                            nc.sync.dma_start(out=outT[m * P:(m + 1) * P, :],
                                              in_=ev)

                heads_rhs = {}
                for h in range(HPC):
                    heads_rhs[h] = attention_head(h)
                    if h >= 1:
                        wo_pass(h - 1, *heads_rhs[h - 1])
                wo_pass(HPC - 1, *heads_rhs[HPC - 1])

    nc.compile()
    return nc


def _prep_inputs(x, freqs_cos, freqs_sin, wq_a, q_norm_w, wq_b, wkv_a,
                 kv_norm_w, wkv_b, wo):
    x = np.asarray(x, np.float32)
    freqs_cos = np.asarray(freqs_cos, np.float32)
    freqs_sin = np.asarray(freqs_sin, np.float32)
    wq_a = np.asarray(wq_a, np.float32)
    q_norm_w = np.asarray(q_norm_w, np.float32)
    wq_b = np.asarray(wq_b, np.float32)
    wkv_a = np.asarray(wkv_a, np.float32)
    kv_norm_w = np.asarray(kv_norm_w, np.float32)
    wkv_b = np.asarray(wkv_b, np.float32)
    wo = np.asarray(wo, np.float32)

    wqaT = np.ascontiguousarray(wq_a.T).astype(BF16)
    wkvaT = np.ascontiguousarray(wkv_a.T).astype(BF16)

    wqb_eff = (wq_b * q_norm_w[None, :]) * SCALE
    wqb_eff = wqb_eff.reshape(N_HEADS, QK_HD, Q_LORA)
    wkvb_eff = wkv_b * kv_norm_w[None, :]
    wkvb_eff = wkvb_eff.reshape(N_HEADS, NOPE + V_DIM, KV_LORA)

    cosT = np.tile(np.repeat(freqs_cos.T, 2, axis=0), (2, 1))  # [128, S]
    sinT = np.tile(np.repeat(freqs_sin.T, 2, axis=0), (2, 1))

    perm64_ = np.zeros((ROPE, ROPE), np.float32)
    for i in range(ROPE // 2):
        perm64_[2 * i + 1, 2 * i] = -1.0  # out[2i]   = -x[2i+1]
        perm64_[2 * i, 2 * i + 1] = 1.0   # out[2i+1] =  x[2i]
    perm = np.zeros((P, P), np.float32)
    perm[:ROPE, :ROPE] = perm64_
    perm[ROPE:, ROPE:] = perm64_
    r = np.arange(P)
    trimask = np.where(r[:, None] <= r[None, :], 0.0,
                       -1e30).astype(np.float32)

    # wo.T rows regrouped so pass h contracts head g'*4+h for g'=0..3:
    # woTr rows [h*512 + g'*128 : ...] = wo.T rows of head g'*4+h
    woT4 = wo.T.reshape(N_HEADS // 4, 4, V_DIM, DIM)  # [g', h, 128, D]
    woTr = np.ascontiguousarray(
        woT4.transpose(1, 0, 2, 3).reshape(N_HEADS * V_DIM, DIM)).astype(BF16)

    in_maps = []
    for c in range(NCORES):
        b, g = c // GROUP, c % GROUP
        heads = slice(g * HPC, (g + 1) * HPC)
        xTc = np.ascontiguousarray(
            x[b].T[:, g * SSH:(g + 1) * SSH]).astype(BF16)
        wqbT = np.concatenate(
            [wqb_eff[heads, :NOPE].reshape(HPC * NOPE, Q_LORA),
             wqb_eff[heads, NOPE:].reshape(HPC * ROPE, Q_LORA)],
            axis=0).T
        wkvbT = np.concatenate(
            [wkvb_eff[heads, :NOPE].reshape(HPC * NOPE, KV_LORA),
             wkvb_eff[heads, NOPE:].reshape(HPC * V_DIM, KV_LORA)],
            axis=0).T
        in_maps.append({
            "xT": xTc,
            "wqaT": wqaT,
            "wkvaT": wkvaT,
            "wqbT": np.ascontiguousarray(wqbT).astype(BF16),
            "wkvbT": np.ascontiguousarray(wkvbT).astype(BF16),
            "woTr": woTr,
            "cos_sh": np.ascontiguousarray(
                cosT[:, g * SSH:(g + 1) * SSH]).astype(BF16),
            "sin_sh": np.ascontiguousarray(
                sinT[:, g * SSH:(g + 1) * SSH]).astype(BF16),
            "cos_full": np.ascontiguousarray(cosT).astype(BF16),
            "sin_full": np.ascontiguousarray(sinT).astype(BF16),
            "perm64": perm.astype(BF16),
            "trimask": trimask,
            "cfg": np.array([[b * GROUP * P]], np.int32),
        })
    return in_maps


def _run(inputs, trace=False, **kw):
    if "nc" not in _cache:
        _cache["nc"] = _build()
    nc = _cache["nc"]
    in_maps = _prep_inputs(**inputs)
    res = run_bass_kernel_spmd(nc, in_maps, core_ids=list(range(NCORES)),
                               trace=trace, **kw)
    out = np.empty((B, S, DIM), np.float32)
    for c in range(NCORES):
        b, g = c // GROUP, c % GROUP
        out[b, g * SSH:(g + 1) * SSH, :] = res.results[c]["out"].T
    return out, res


def kernel(**inputs):
    out, _ = _run(inputs)
    return out


# revision 33
# speedup vs baseline: 1.1611x; 1.1611x over previous
"""MLA attention (DeepSeek-style) distributed over 8 TRN2 NeuronCores.

Sharding: core c -> batch b=c//4, head-group/seq-shard g=c%4.
Phase A: down-proj of own 512-pos shard -> bf16 bounce -> group-of-4
AllGathers (kv first, then q split in two, so comms overlap projection
compute). kv latent is pre-scaled by inv-rms before the bounce.
Phase B: up-proj (Qt/Kt d-major, V row-major), flash-style causal attention
with St[kv,q] layout (softmax denominators via ones-matmul, no transposes).
Attention outputs exchanged per head via an 8-rank AllToAll (each rank's
shard j carries the o block destined for rank j); the row-parallel wo
matmul on the own s-shard is interleaved between attention heads and
accumulated in SBUF.
"""

import numpy as np
import ml_dtypes

import concourse.bass as bass
import concourse.bacc as bacc
import concourse.tile as tile
import concourse.mybir as mybir
from concourse.bass_utils import run_bass_kernel_spmd

BF16 = ml_dtypes.bfloat16

# problem constants (hardcoded per harness rules)
DIM = 2048
N_HEADS = 16
Q_LORA = 1536
KV_LORA = 512
NOPE = 128
ROPE = 64
V_DIM = 128
QK_HD = NOPE + ROPE  # 192
EPS = 1e-6
B, S = 2, 2048
SCALE = QK_HD ** -0.5

NCORES = 8
GROUP = 4               # cores per batch
SSH = S // GROUP        # 512, seq shard
HPC = N_HEADS // GROUP  # 4 heads per core
P = 128
NKT = DIM // P          # 16
NQM = Q_LORA // P       # 12
NKVM = KV_LORA // P     # 4
NCH = S // 512          # 4
KVROWS = KV_LORA + ROPE       # 576: pre-scaled kv latent + rope'd k_pe
                              # (k_pe is ready early, and the last q AG gates
                              # the critical path, so it rides with kv)
NQ1 = 7                       # q slabs in first AG (rebalanced: the second
                              # AG is chained on the first's completion, so
                              # shrinking it ends the chain sooner; both
                              # stay <1MB per-rank to keep the Mesh algo)
Q1ROWS = NQ1 * P              # 896
Q2ROWS = (NQM - NQ1) * P + 1  # 641: q slabs 7-11 + a_q row

_cache = {}


def _build():
    nc = bacc.Bacc("TRN2", target_bir_lowering=False, debug=False,
                   num_devices=NCORES)
    f32 = mybir.dt.float32
    bf = mybir.dt.bfloat16
    i32 = mybir.dt.int32

    # ---- dram parameters ----
    xT = nc.dram_tensor("xT", [DIM, SSH], bf, kind="ExternalInput")
    wqaT = nc.dram_tensor("wqaT", [DIM, Q_LORA], bf, kind="ExternalInput")
    wkvaT = nc.dram_tensor("wkvaT", [DIM, KV_LORA + ROPE], bf,
                           kind="ExternalInput")
    wqbT = nc.dram_tensor("wqbT", [Q_LORA, HPC * QK_HD], bf,
                          kind="ExternalInput")
    wkvbT = nc.dram_tensor("wkvbT", [KV_LORA, HPC * (NOPE + V_DIM)], bf,
                           kind="ExternalInput")
    # wo.T rows regrouped head-major: rows [h*512 + g'*128 ...] = head g'*4+h
    woTr = nc.dram_tensor("woTr", [N_HEADS * V_DIM, DIM], bf,
                          kind="ExternalInput")
    cos_sh = nc.dram_tensor("cos_sh", [P, SSH], bf, kind="ExternalInput")
    sin_sh = nc.dram_tensor("sin_sh", [P, SSH], bf, kind="ExternalInput")
    cos_full = nc.dram_tensor("cos_full", [P, S], bf, kind="ExternalInput")
    sin_full = nc.dram_tensor("sin_full", [P, S], bf, kind="ExternalInput")
    perm64 = nc.dram_tensor("perm64", [P, P], bf, kind="ExternalInput")
    trimask = nc.dram_tensor("trimask", [P, P], f32, kind="ExternalInput")
    cfg = nc.dram_tensor("cfg", [1, 1], i32, kind="ExternalInput")
    outT = nc.dram_tensor("out", [DIM, SSH], f32, kind="ExternalOutput")

    # ---- internal dram ----
    b_kv = nc.dram_tensor("b_kv", [KVROWS, SSH], bf)
    g_kv = nc.dram_tensor("g_kv", [GROUP * KVROWS, SSH], bf)
    b_q1 = nc.dram_tensor("b_q1", [Q1ROWS, SSH], bf)
    g_q1 = nc.dram_tensor("g_q1", [GROUP * Q1ROWS, SSH], bf)
    b_q2 = nc.dram_tensor("b_q2", [Q2ROWS, SSH], bf)
    g_q2 = nc.dram_tensor("g_q2", [GROUP * Q2ROWS, SSH], bf)
    # AllToAll o exchange: shard j (rows j*128..) goes to rank j. Only the
    # own-batch shards carry data; other-batch shards are never read.
    o_in = [nc.dram_tensor(f"o_in{h}", [NCORES * V_DIM, SSH], bf)
            for h in range(HPC)]
    o_out = [nc.dram_tensor(f"o_out{h}", [NCORES * V_DIM, SSH], bf)
             for h in range(HPC)]
    rg4 = [[0, 1, 2, 3], [4, 5, 6, 7]]
    rg8 = [list(range(NCORES))]

    with tile.TileContext(nc) as tc:
        with (
            tc.tile_pool(name="persist", bufs=1) as persist,
            tc.tile_pool(name="attn", bufs=1) as attn_pool,
            tc.tile_pool(name="wts", bufs=1) as wts,
        ):
            # constants (cheap, engine-local)
            ones_f = persist.tile([P, 1], f32)
            nc.vector.memset(ones_f, 1.0)
            ones_b = persist.tile([P, 1], bf)
            nc.vector.memset(ones_b, 1.0)
            ones_row = persist.tile([1, P], bf)
            nc.vector.memset(ones_row, 1.0)
            eps_sb = persist.tile([1, 1], f32)
            nc.vector.memset(eps_sb, EPS)
            cfg_sb = persist.tile([1, 1], i32)
            nc.sync.dma_start(out=cfg_sb, in_=cfg[:])

            # per-core dynamic offset: ob = b*512 (A2A shard base)
            r0 = nc.alloc_registers()
            nc.regs_load(r0, cfg_sb[0:1, 0:1])
            ob = nc.snap(r0, donate=True, min_val=0, max_val=GROUP * P)

            # attention-phase persistent tiles (filled by up-proj)
            qt_nope = [attn_pool.tile([P, S], bf, tag=f"qtn{h}",
                                      name=f"qt_nope{h}") for h in range(HPC)]
            qt_pe = [attn_pool.tile([P, S], bf, tag=f"qtp{h}",
                                    name=f"qt_pe{h}")
                     for h in range(HPC // 2)]
            kt_nope = [attn_pool.tile([P, S], bf, tag=f"ktn{h}",
                                      name=f"kt_nope{h}") for h in range(HPC)]
            v_all = attn_pool.tile([P, S // P, HPC * V_DIM], bf)
            kpe_dup = attn_pool.tile([P, NCH, 512], bf)

            # weights/tables prefetched on the gpsimd SWDGE queue (idle
            # early) so they don't delay phase-A HWDGE traffic
            cos_sh_sb = persist.tile([P, SSH], bf)
            nc.gpsimd.dma_start(out=cos_sh_sb, in_=cos_sh[:])
            sin_sh_sb = persist.tile([P, SSH], bf)
            nc.gpsimd.dma_start(out=sin_sh_sb, in_=sin_sh[:])
            perm_sb = persist.tile([P, P], bf)
            nc.gpsimd.dma_start(out=perm_sb, in_=perm64[:])
            wkvb = wts.tile([P, NKVM, HPC * (NOPE + V_DIM)], bf)
            nc.gpsimd.dma_start(
                out=wkvb, in_=wkvbT[:].rearrange("(kt p) m -> p kt m", p=P))
            wqb = wts.tile([P, NQM, HPC * QK_HD], bf)
            nc.gpsimd.dma_start(
                out=wqb, in_=wqbT[:].rearrange("(kt p) m -> p kt m", p=P))
            mask_sb = persist.tile([P, P], f32)
            nc.gpsimd.dma_start(out=mask_sb, in_=trimask[:])
            cos_f_sb = persist.tile([P, S], bf)
            nc.gpsimd.dma_start(out=cos_f_sb, in_=cos_full[:])
            sin_f_sb = persist.tile([P, S], bf)
            nc.gpsimd.dma_start(out=sin_f_sb, in_=sin_full[:])

            # ======== Phase A + up-proj (shared latent pool) ========
            up_lat_cm = tc.tile_pool(name="up_lat", bufs=1)
            up_lat = up_lat_cm.__enter__()
            with (
                tc.tile_pool(name="pa", bufs=3) as pa,
                tc.tile_pool(name="pa_x", bufs=1) as pa_x,
                tc.tile_pool(name="pa_out", bufs=3) as pa_out,
                tc.tile_pool(name="pa_ps", bufs=2, space="PSUM") as pa_ps,
                tc.tile_pool(name="pa_st", bufs=1, space="PSUM") as pa_st,
            ):
                # 8-way split so the first slab's k-loop chases x arrival
                x_all = pa_x.tile([P, NKT, SSH], bf)
                for xq in range(8):
                    nc.sync.dma_start(
                        out=x_all[:, xq * 2:(xq + 1) * 2, :],
                        in_=xT[xq * 2 * P:(xq + 1) * 2 * P, :].rearrange(
                            "(kt p) s -> p kt s", p=P))

                q_stat = pa_st.tile([1, SSH], f32)
                kv_stat = pa_st.tile([1, SSH], f32)

                def down_slab(wT, m0, mrows, ev, stat_ps, stat_first,
                              stat_last):
                    slab = pa.tile([P, NKT, mrows], bf, tag="slab")
                    nc.scalar.dma_start(
                        out=slab,
                        in_=wT[:, m0:m0 + mrows].rearrange(
                            "(kt p) m -> p kt m", p=P))
                    ps = pa_ps.tile([P, SSH], f32, tag="dps")
                    for k in range(NKT):
                        nc.tensor.matmul(ps[:mrows, :], slab[:, k, :],
                                         x_all[:, k, :], start=(k == 0),
                                         stop=(k == NKT - 1))
                    nc.vector.tensor_copy(ev[:mrows, :], ps[:mrows, :])
                    if stat_ps is not None:
                        # square on the vector engine (from the SBUF copy:
                        # DVE reads at most one PSUM operand): the scalar
                        # queue must stay pure-DMA-dispatch or slab loads
                        # serialize behind psum-dependent compute
                        sq = pa.tile([P, SSH], f32, tag="sq")
                        nc.vector.tensor_mul(sq[:mrows, :], ev[:mrows, :],
                                             ev[:mrows, :])
                        nc.tensor.matmul(stat_ps, ones_f[:mrows, :],
                                         sq[:mrows, :], start=stat_first,
                                         stop=stat_last)

                def inv_rms(stat, n):
                    tmp = pa.tile([1, SSH], f32, tag="srt")
                    nc.scalar.activation(tmp, stat,
                                         mybir.ActivationFunctionType.Sqrt,
                                         bias=eps_sb[0:1, 0:1], scale=1.0 / n)
                    rcp = pa.tile([1, SSH], f32, tag="rcp")
                    nc.vector.reciprocal(rcp, tmp)
                    rb = pa.tile([1, SSH], bf, tag="rb")
                    nc.vector.tensor_copy(rb, rcp)
                    return rb

                # ---- kv first (so AG_kv overlaps q down-proj) ----
                kv_ev = pa_x.tile([P, NKVM, SSH], bf)
                for m in range(NKVM):
                    down_slab(wkvaT, m * P, P, kv_ev[:, m, :], kv_stat,
                              m == 0, m == NKVM - 1)
                # rope'd k_pe rides in the q AllGather (not needed until
                # attention), keeping AG_kv minimal on the critical path
                kpe_ev = pa_out.tile([P, SSH], bf, tag="kpe_ev")
                down_slab(wkvaT, KV_LORA, ROPE, kpe_ev, None, False, False)
                xs_ps = pa_ps.tile([ROPE, SSH], f32, tag="xs")
                nc.tensor.matmul(xs_ps, perm_sb[:ROPE, :ROPE], kpe_ev[:ROPE, :])
                y0 = pa.tile([ROPE, SSH], bf, tag="ry0")
                nc.vector.tensor_mul(y0, kpe_ev[:ROPE, :], cos_sh_sb[:ROPE, :])
                y1 = pa.tile([ROPE, SSH], bf, tag="ry1")
                nc.vector.tensor_mul(y1, xs_ps, sin_sh_sb[:ROPE, :])
                yr = pa.tile([ROPE, SSH], bf, tag="ryr")
                nc.vector.tensor_add(yr, y0, y1)
                nc.sync.dma_start(out=b_kv[KV_LORA:KVROWS, :], in_=yr)
                # first q slabs keep the tensor stream dense while the kv
                # stat -> scale -> bounce chain resolves
                for m in range(2):
                    ev = pa_out.tile([P, SSH], bf, tag="qev")
                    down_slab(wqaT, m * P, P, ev, q_stat, m == 0, False)
                    nc.sync.dma_start(out=b_q1[m * P:(m + 1) * P, :], in_=ev)
                # pre-scale kv latent by inv-rms, then bounce. The
                # partition-broadcast of the inv-rms row is a K=1 matmul
                # (ones column outer product) -- the gpsimd broadcast costs
                # a lib load plus ~15us of false semaphore waits here.
                rkv = inv_rms(kv_stat, KV_LORA)
                rkv_ps = pa_ps.tile([P, SSH], f32, tag="rkb")
                nc.tensor.matmul(rkv_ps, ones_row, rkv)
                for m in range(NKVM):
                    nc.vector.tensor_mul(kv_ev[:, m, :], kv_ev[:, m, :],
                                         rkv_ps)
                    nc.sync.dma_start(out=b_kv[m * P:(m + 1) * P, :],
                                      in_=kv_ev[:, m, :])

                nc.gpsimd.collective_compute(
                    "AllGather", mybir.AluOpType.bypass, replica_groups=rg4,
                    ins=[b_kv[:]], outs=[g_kv[:]])

                # kv gather loads on the gpsimd queue right behind the AG_kv
                # trigger: gpsimd SWDGE uses its own semaphore lanes, so
                # these AG-gated loads cannot falsely delay the HWDGE
                # bounce-write lanes that gate the later AG triggers
                kv_lat = up_lat.tile([P, NKVM, NCH, 512], bf)
                for r in range(NCH):
                    nc.gpsimd.dma_start(
                        out=kv_lat[:, :, r, :],
                        in_=g_kv[r * KVROWS:r * KVROWS + KV_LORA, :]
                        .rearrange("(kt p) s -> p kt s", p=P))

                # ---- remaining q down-proj (overlaps AG_kv / AG_q1) ----
                for m in range(2, NQ1):
                    ev = pa_out.tile([P, SSH], bf, tag="qev")
                    down_slab(wqaT, m * P, P, ev, q_stat, False, False)
                    nc.sync.dma_start(out=b_q1[m * P:(m + 1) * P, :], in_=ev)
                nc.gpsimd.collective_compute(
                    "AllGather", mybir.AluOpType.bypass, replica_groups=rg4,
                    ins=[b_q1[:]], outs=[g_q1[:]])

                for m in range(NQ1, NQM):
                    ev = pa_out.tile([P, SSH], bf, tag="qev")
                    down_slab(wqaT, m * P, P, ev, q_stat, False,
                              m == NQM - 1)
                    nc.sync.dma_start(
                        out=b_q2[(m - NQ1) * P:(m - NQ1 + 1) * P, :], in_=ev)
                rq = inv_rms(q_stat, Q_LORA)
                nc.sync.dma_start(
                    out=b_q2[(NQM - NQ1) * P:(NQM - NQ1) * P + 1, :], in_=rq)
                nc.gpsimd.collective_compute(
                    "AllGather", mybir.AluOpType.bypass, replica_groups=rg4,
                    ins=[b_q2[:]], outs=[g_q2[:]])

                # kpe / a_q gather loads, also on the gpsimd lanes
                aq_row = up_lat.tile([1, NCH, 512], bf)
                for r in range(NCH):
                    nc.gpsimd.dma_start(
                        out=kpe_dup[:ROPE, r, :],
                        in_=g_kv[r * KVROWS + KV_LORA:(r + 1) * KVROWS, :])
                    nc.gpsimd.dma_start(
                        out=kpe_dup[ROPE:, r, :],
                        in_=g_kv[r * KVROWS + KV_LORA:(r + 1) * KVROWS, :])
                    nc.gpsimd.dma_start(
                        out=aq_row[0:1, r, :],
                        in_=g_q2[r * Q2ROWS + (NQM - NQ1) * P:
                                 r * Q2ROWS + (NQM - NQ1) * P + 1, :])
                a_q_bc = up_lat.tile([P, NCH, 512], bf)
                for r in range(NCH):
                    nc.gpsimd.partition_broadcast(a_q_bc[:, r, :],
                                                  aq_row[0:1, r, :])

            # ================= Phase B: up projections =================
            with (
                tc.tile_pool(name="up", bufs=3) as up,
                tc.tile_pool(name="qlat", bufs=2) as qlat_pool,
                tc.tile_pool(name="up_ps", bufs=3, space="PSUM") as up_ps,
                tc.tile_pool(name="pe_ps", bufs=2, space="PSUM") as pe_ps,
            ):
                # k_nope (d-major) + v (row-major); kv_lat is pre-scaled
                for c in range(NCH):
                    for h in range(HPC):
                        ps = up_ps.tile([P, 512], f32, tag="up")
                        for k in range(NKVM):
                            nc.tensor.matmul(
                                ps, wkvb[:, k, h * NOPE:(h + 1) * NOPE],
                                kv_lat[:, k, c, :], start=(k == 0),
                                stop=(k == NKVM - 1))
                        nc.vector.tensor_copy(
                            kt_nope[h][:, c * 512:(c + 1) * 512], ps)
                for sb in range(S // P):
                    c, part = sb // 4, sb % 4
                    ps = up_ps.tile([P, HPC * V_DIM], f32, tag="up")
                    for k in range(NKVM):
                        nc.tensor.matmul(
                            ps, kv_lat[:, k, c, part * P:(part + 1) * P],
                            wkvb[:, k, HPC * NOPE:], start=(k == 0),
                            stop=(k == NKVM - 1))
                    nc.vector.tensor_copy(v_all[:, sb, :], ps)

                # ---- q up-proj (waits on AG_q1/2; q_lat streamed) ----
                for c in range(NCH):
                    ql = qlat_pool.tile([P, NQM, 512], bf, tag="ql",
                                        name="ql")
                    nc.sync.dma_start(
                        out=ql[:, 0:NQ1, :],
                        in_=g_q1[c * Q1ROWS:(c + 1) * Q1ROWS, :]
                        .rearrange("(kt p) s -> p kt s", p=P))
                    nc.sync.dma_start(
                        out=ql[:, NQ1:, :],
                        in_=g_q2[c * Q2ROWS:c * Q2ROWS + (NQM - NQ1) * P, :]
                        .rearrange("(kt p) s -> p kt s", p=P))
                    for h in range(HPC):
                        ps = up_ps.tile([P, 512], f32, tag="up")
                        for k in range(NQM):
                            nc.tensor.matmul(
                                ps, wqb[:, k, h * P:(h + 1) * P],
                                ql[:, k, :], start=(k == 0),
                                stop=(k == NQM - 1))
                        nc.vector.tensor_mul(
                            qt_nope[h][:, c * 512:(c + 1) * 512], ps,
                            a_q_bc[:, c, :])
                    for hp in range(HPC // 2):
                        pcol0 = HPC * NOPE + 2 * hp * ROPE
                        ps = pe_ps.tile([P, 512], f32, tag="qp")
                        for k in range(NQM):
                            nc.tensor.matmul(
                                ps, wqb[:, k, pcol0:pcol0 + 2 * ROPE],
                                ql[:, k, :], start=(k == 0),
                                stop=(k == NQM - 1))
                        pe_s = up.tile([P, 512], bf, tag="pes")
                        nc.vector.tensor_mul(pe_s, ps, a_q_bc[:, c, :])
                        xs = pe_ps.tile([P, 512], f32, tag="qpx")
                        nc.tensor.matmul(xs, perm_sb, pe_s)
                        dst = qt_pe[hp][:, c * 512:(c + 1) * 512]
                        nc.vector.tensor_mul(
                            dst, pe_s, cos_f_sb[:, c * 512:(c + 1) * 512])
                        t1 = up.tile([P, 512], bf, tag="pet")
                        nc.vector.tensor_mul(
                            t1, xs, sin_f_sb[:, c * 512:(c + 1) * 512])
                        nc.vector.tensor_add(dst, dst, t1)

            up_lat_cm.__exit__(None, None, None)

            # ========== attention + per-head A2As + interleaved wo ==========
            with (
                tc.tile_pool(name="at", bufs=3) as at,
                tc.tile_pool(name="at_rl", bufs=2) as at_rl,
                tc.tile_pool(name="wo_rhs", bufs=2) as wo_rhs,
                tc.tile_pool(name="wo_acc", bufs=1) as wo_acc,
                tc.tile_pool(name="wo_w", bufs=2) as wo_w,
                tc.tile_pool(name="wo_ev", bufs=3) as wo_ev,
                tc.tile_pool(name="st_ps", bufs=2, space="PSUM") as st_ps,
                tc.tile_pool(name="ot_ps", bufs=2, space="PSUM") as ot_ps,
                tc.tile_pool(name="l_ps", bufs=2, space="PSUM") as l_ps,
                tc.tile_pool(name="wo_ps", bufs=2, space="PSUM") as wo_ps,
            ):
                acc = wo_acc.tile([P, NKT, 512], f32)

                def attention_head(h):
                    pending = None  # (pj, off, j, ot, lt, first, last)

                    def flush():
                        nonlocal pending
                        if pending is None:
                            return
                        pj, off, j, ot, lt, first, last = pending
                        nc.tensor.matmul(lt[:, off:], ones_b, pj[:, off:],
                                         start=first, stop=last)
                        nc.tensor.matmul(
                            ot[:, off:],
                            v_all[:, j, h * V_DIM:(h + 1) * V_DIM],
                            pj[:, off:], start=first, stop=last)
                        pending = None

                    def evict(ot, lt, qc):
                        rl = at_rl.tile([1, 512], f32, tag="rl", name="rl")
                        nc.vector.reciprocal(rl, lt)
                        rlb = at_rl.tile([P, 512], f32, tag="rlb",
                                         name="rlb")
                        nc.gpsimd.partition_broadcast(rlb, rl)
                        ev = at.tile([P, 512], bf, tag="oev", name="oev")
                        nc.vector.tensor_mul(ev, ot, rlb)
                        nc.sync.dma_start(
                            out=o_in[h][bass.ds(ob + qc * P, P), :],
                            in_=ev)

                    # prefetch this head's wo weights (no deps)
                    wslab = wo_w.tile([P, GROUP, DIM], bf, tag="woslab",
                                      name="wslab")
                    nc.scalar.dma_start(
                        out=wslab,
                        in_=woTr[h * 512:(h + 1) * 512, :].rearrange(
                            "(kt p) m -> p kt m", p=P))

                    evs = []
                    for qc in range(NCH):
                        nj = qc * 4 + 4
                        ot = ot_ps.tile([P, 512], f32, tag="ot", name="ot")
                        lt = l_ps.tile([1, 512], f32, tag="l", name="lt")
                        for j in range(nj):
                            d = j - qc * 4
                            off = max(0, d) * P
                            st = st_ps.tile([P, 512], f32, tag="st",
                                            name="st")
                            nc.tensor.matmul(
                                st[:, off:],
                                kt_nope[h][:, j * P:(j + 1) * P],
                                qt_nope[h][:, qc * 512 + off:(qc + 1) * 512],
                                start=True, stop=False)
                            lo = (h % 2) * ROPE
                            nc.tensor.matmul(
                                st[:, off:],
                                kpe_dup[lo:lo + ROPE, j // 4,
                                        (j % 4) * P:(j % 4 + 1) * P],
                                qt_pe[h // 2][lo:lo + ROPE,
                                              qc * 512 + off:(qc + 1) * 512],
                                start=False, stop=True)
                            flush()
                            if j == 0 and evs:
                                # previous chunk's accumulation closed with
                                # the flush above: evict it now so the o
                                # exchange isn't serialized at head end
                                evict(*evs.pop())
                            if d >= 0:
                                nc.vector.tensor_add(st[:, off:off + P],
                                                     st[:, off:off + P],
                                                     mask_sb)
                            pj = at.tile([P, 512], bf, tag="p", name="pj")
                            nc.scalar.activation(
                                pj[:, off:], st[:, off:],
                                mybir.ActivationFunctionType.Exp)
                            pending = (pj, off, j, ot, lt, j == 0,
                                       j == nj - 1)
                        evs.append((ot, lt, qc))
                    flush()
                    evict(*evs.pop())
                    nc.gpsimd.collective_compute(
                        "AllToAll", mybir.AluOpType.bypass,
                        replica_groups=rg8, ins=[o_in[h][:]],
                        outs=[o_out[h][:]])
                    # rhs loads for the wo pass of this head (gpsimd queue,
                    # blocked only by this A2A)
                    rhs = wo_rhs.tile([P, GROUP, 512], bf, tag="rhs",
                                      name="rhs")
                    nc.gpsimd.dma_start(
                        out=rhs,
                        in_=o_out[h][bass.ds(ob, GROUP * P), :].rearrange(
                            "(k p) s -> p k s", p=P))
                    return rhs, wslab

                def wo_pass(h, rhs, wslab):
                    for m in range(NKT):
                        ps = wo_ps.tile([P, 512], f32, tag="wops",
                                        name="wops")
                        for k in range(GROUP):
                            nc.tensor.matmul(
                                ps, wslab[:, k, m * P:(m + 1) * P],
                                rhs[:, k, :], start=(k == 0),
                                stop=(k == GROUP - 1))
                        if h == 0:
                            nc.vector.tensor_copy(acc[:, m, :], ps)
                        elif h < HPC - 1:
                            nc.vector.tensor_add(acc[:, m, :], ps,
                                                 acc[:, m, :])
                        else:
                            ev = wo_ev.tile([P, 512], f32, tag="woev",
                                            name="woev")
                            nc.vector.tensor_add(ev, ps, acc[:, m, :])
                            nc.sync.dma_start(out=outT[m * P:(m + 1) * P, :],
                                              in_=ev)

                heads_rhs = {}
                for h in range(HPC):
                    heads_rhs[h] = attention_head(h)
                    if h >= 1:
                        wo_pass(h - 1, *heads_rhs[h - 1])
                wo_pass(HPC - 1, *heads_rhs[HPC - 1])

    nc.compile()
    return nc


def _prep_inputs(x, freqs_cos, freqs_sin, wq_a, q_norm_w, wq_b, wkv_a,
                 kv_norm_w, wkv_b, wo):
    x = np.asarray(x, np.float32)
    freqs_cos = np.asarray(freqs_cos, np.float32)
    freqs_sin = np.asarray(freqs_sin, np.float32)
    wq_a = np.asarray(wq_a, np.float32)
    q_norm_w = np.asarray(q_norm_w, np.float32)
    wq_b = np.asarray(wq_b, np.float32)
    wkv_a = np.asarray(wkv_a, np.float32)
    kv_norm_w = np.asarray(kv_norm_w, np.float32)
    wkv_b = np.asarray(wkv_b, np.float32)
    wo = np.asarray(wo, np.float32)

    wqaT = np.ascontiguousarray(wq_a.T).astype(BF16)
    wkvaT = np.ascontiguousarray(wkv_a.T).astype(BF16)

    wqb_eff = (wq_b * q_norm_w[None, :]) * SCALE
    wqb_eff = wqb_eff.reshape(N_HEADS, QK_HD, Q_LORA)
    wkvb_eff = wkv_b * kv_norm_w[None, :]
    wkvb_eff = wkvb_eff.reshape(N_HEADS, NOPE + V_DIM, KV_LORA)

    cosT = np.tile(np.repeat(freqs_cos.T, 2, axis=0), (2, 1))  # [128, S]
    sinT = np.tile(np.repeat(freqs_sin.T, 2, axis=0), (2, 1))

    perm64_ = np.zeros((ROPE, ROPE), np.float32)
    for i in range(ROPE // 2):
        perm64_[2 * i + 1, 2 * i] = -1.0  # out[2i]   = -x[2i+1]
        perm64_[2 * i, 2 * i + 1] = 1.0   # out[2i+1] =  x[2i]
    perm = np.zeros((P, P), np.float32)
    perm[:ROPE, :ROPE] = perm64_
    perm[ROPE:, ROPE:] = perm64_
    r = np.arange(P)
    trimask = np.where(r[:, None] <= r[None, :], 0.0,
                       -1e30).astype(np.float32)

    # wo.T rows regrouped so pass h contracts head g'*4+h for g'=0..3:
    # woTr rows [h*512 + g'*128 : ...] = wo.T rows of head g'*4+h
    woT4 = wo.T.reshape(N_HEADS // 4, 4, V_DIM, DIM)  # [g', h, 128, D]
    woTr = np.ascontiguousarray(
        woT4.transpose(1, 0, 2, 3).reshape(N_HEADS * V_DIM, DIM)).astype(BF16)

    in_maps = []
    for c in range(NCORES):
        b, g = c // GROUP, c % GROUP
        heads = slice(g * HPC, (g + 1) * HPC)
        xTc = np.ascontiguousarray(
            x[b].T[:, g * SSH:(g + 1) * SSH]).astype(BF16)
        wqbT = np.concatenate(
            [wqb_eff[heads, :NOPE].reshape(HPC * NOPE, Q_LORA),
             wqb_eff[heads, NOPE:].reshape(HPC * ROPE, Q_LORA)],
            axis=0).T
        wkvbT = np.concatenate(
            [wkvb_eff[heads, :NOPE].reshape(HPC * NOPE, KV_LORA),
             wkvb_eff[heads, NOPE:].reshape(HPC * V_DIM, KV_LORA)],
            axis=0).T
        in_maps.append({
            "xT": xTc,
            "wqaT": wqaT,
            "wkvaT": wkvaT,
            "wqbT": np.ascontiguousarray(wqbT).astype(BF16),
            "wkvbT": np.ascontiguousarray(wkvbT).astype(BF16),
            "woTr": woTr,
            "cos_sh": np.ascontiguousarray(
                cosT[:, g * SSH:(g + 1) * SSH]).astype(BF16),
            "sin_sh": np.ascontiguousarray(
                sinT[:, g * SSH:(g + 1) * SSH]).astype(BF16),
            "cos_full": np.ascontiguousarray(cosT).astype(BF16),
            "sin_full": np.ascontiguousarray(sinT).astype(BF16),
            "perm64": perm.astype(BF16),
            "trimask": trimask,
            "cfg": np.array([[b * GROUP * P]], np.int32),
        })
    return in_maps


def _run(inputs, trace=False, **kw):
    if "nc" not in _cache:
        _cache["nc"] = _build()
    nc = _cache["nc"]
    in_maps = _prep_inputs(**inputs)
    res = run_bass_kernel_spmd(nc, in_maps, core_ids=list(range(NCORES)),
                               trace=trace, **kw)
    out = np.empty((B, S, DIM), np.float32)
    for c in range(NCORES):
        b, g = c // GROUP, c % GROUP
        out[b, g * SSH:(g + 1) * SSH, :] = res.results[c]["out"].T
    return out, res


def kernel(**inputs):
    out, _ = _run(inputs)
    return out
